# revision 74
# baseline (speedup 1.0000x reference)
"""Multi-head self-attention with LoRA on 8 Trainium2 NeuronCores.

Sharding: core c -> (batch b = c//2, head-half = c%2): each core computes
8 of the 16 heads for one batch (tensor parallel over heads), then a
partial O-projection over its 1024 input dims. Host sums the two partial
O outputs per batch and adds the O bias (gather-reduce unshard).

Per core (all SBUF-resident, bf16 matmul inputs, fp32 PSUM accumulate):
  - xT [din, tok] arrives pre-transposed from host (bf16)
  - z = (SCALING*B) @ xT for q/k/v in one combined 48-wide pass
  - per head: qT/kT/vT projections (LoRA + bias folded into the psum
    accumulation chain via an extra rank-17 matmul), v re-transposed to
    natural layout with an identity-moving matmul
  - attention: scores -> exp (Act engine, scale folded) -> ones-matmul
    denominators -> attn@v -> reciprocal-normalize
  - software pipelining: head h's attention is interleaved with head
    h+1's projections so PE never waits on the Act engine's exp
  - partial O-projection [2048, tok] over this core's 1024 dims
Host: input packing/transposes/casts and output pair-sum + bias.
"""

import os
import numpy as np
import ml_dtypes

import concourse.bacc as bacc
import concourse.mybir as mybir
import concourse.tile as tile
from concourse.bass_utils import run_bass_kernel_spmd

F32 = mybir.dt.float32
F32R = mybir.dt.float32r
BF16 = mybir.dt.bfloat16
AF = mybir.ActivationFunctionType
NPBF = ml_dtypes.bfloat16

B, L, D = 4, 2048, 2048
H, HD, R = 16, 128, 16
HC = H // 2            # 8 heads per core
DS = HC * HD           # 1024-dim q/k/v output slice per core
SCALING = 0.5          # lora alpha / rank (folded into B on host)
SCALE = HD ** -0.5     # attention score scale (folded into exp)
P = 128
KI = D // P            # 16 contraction tiles (full model dim)
KO = DS // P           # 8 contraction tiles (this core's O input slice)
NT = D // P            # 16 output tiles for O projection
TT = L // P            # 16 key tiles
CH = 512               # moving-dim chunk
NCH = L // CH          # 4 chunks of query tokens
R1 = R + 1             # lora rank + bias row
NCORES = 8

_cache = {}


def _build():
    nc = bacc.Bacc()

    xT = nc.dram_tensor("xT", [D, L], BF16, kind="ExternalInput")
    # staged weight layouts (packed on host): [do, p, ki, f]
    wq = nc.dram_tensor("wq", [HC, P, KI, P], BF16, kind="ExternalInput")
    wk = nc.dram_tensor("wk", [HC, P, KI, P], BF16, kind="ExternalInput")
    wv = nc.dram_tensor("wv", [HC, P, KI, P], BF16, kind="ExternalInput")
    wo = nc.dram_tensor("wo", [NT, P, KO, P], BF16, kind="ExternalInput")
    # combined scaled B^T for q/k/v, padded to 32-wide blocks: [p, ki, 96]
    zb = nc.dram_tensor("zb", [P, KI, 96], BF16, kind="ExternalInput")
    # scaled B_o^T slice: [p, ki(8), 16]
    bz = nc.dram_tensor("bz", [P, KO, R], BF16, kind="ExternalInput")
    # A^T slices with bias rows, at 32-aligned base partitions per proj:
    # rows 32p..32p+15 = A_p^T slice, row 32p+16 = bias_p slice
    abt = nc.dram_tensor("abt", [96, DS], BF16, kind="ExternalInput")
    aoa = nc.dram_tensor("aoa", [R, D], BF16, kind="ExternalInput")
    yt = nc.dram_tensor("yt", [D, L], BF16, kind="ExternalOutput")

    ident_d = nc.inline_tensor(np.eye(P, dtype=NPBF), name="ident_d")
    ones_d = nc.inline_tensor(np.ones((P, P), dtype=NPBF), name="ones_d")

    def dma(out, in_):
        nc.sync.dma_start(out=out, in_=in_)

    def dma_act(out, in_):
        # second hardware DMA queue (Activation engine HWDGE)
        nc.scalar.dma_start(out=out, in_=in_)

    def fr(ap):
        return ap.bitcast(F32R)

    with tile.TileContext(nc) as tc:
        with (
            tc.tile_pool(name="consts", bufs=1) as consts,
            tc.tile_pool(name="data", bufs=1) as data,
            tc.tile_pool(name="hq", bufs=2) as hqpool,
            tc.tile_pool(name="hk", bufs=2) as hkpool,
            tc.tile_pool(name="hv", bufs=2) as hvpool,
            tc.tile_pool(name="wst", bufs=3) as wst,
            tc.tile_pool(name="wost", bufs=3) as wost,
            tc.tile_pool(name="ex", bufs=1) as expool,
            tc.tile_pool(name="sm", bufs=2) as smpool,
            tc.tile_pool(name="pmm", bufs=2, space="PSUM") as pmm,
            tc.tile_pool(name="psc", bufs=2, space="PSUM") as psc,
            tc.tile_pool(name="pso", bufs=1, space="PSUM") as pso,
            tc.tile_pool(name="psr", bufs=1, space="PSUM") as psr,
        ):
            # ---- constants (only what the z pass needs before xT) ----
            ident = consts.tile([P, P], BF16, tag="ident")
            dma(ident, ident_d[:, :])
            ones = consts.tile([P, P], BF16, tag="ones")
            dma(ones, ones_d[:, :])
            zbs = consts.tile([P, KI, 96], BF16, tag="zbs")
            dma(zbs, zb[:, :, :])

            # lora moving operands: rows 32p..32p+15 = z_p, row 32p+16 = ones
            # (32-aligned base partitions as the PE and BIR verifier require)
            zt = consts.tile([96, L], BF16, tag="zt")
            zos = consts.tile([R, L], BF16, tag="zos")

            # big resident tensors; xT DMA'd chunk-major so the prologue's
            # z/proj chains can start before the whole 8MB has landed
            xTs = data.tile([P, KI, L], BF16, tag="xTs")
            # per-head ao tiles: exact tile-level deps let the z_o chain's
            # first 7 matmuls start before head 7's normalize finishes
            ao = [data.tile([P, L], BF16, tag=f"ao{h}", name=f"ao{h}")
                  for h in range(HC)]

            # ---- per-head state ----
            state = {}

            def emit_proj_head_start(h):
                st = {}
                st["ws"] = {}
                for p, wsrc in (("q", wq), ("k", wk), ("v", wv)):
                    ws = wst.tile([P, KI, P], BF16, tag="w", name=f"ws_{p}")
                    dma(ws, wsrc[h, :, :, :])
                    st["ws"][p] = ws
                st["q"] = hqpool.tile([P, L], BF16, tag="qh", name="qh")
                st["k"] = hkpool.tile([P, L], BF16, tag="kh", name="kh")
                st["vT"] = hvpool.tile([P, L], BF16, tag="vTh", name="vTh")
                st["v"] = hvpool.tile([P, TT, P], BF16, tag="vh", name="vh")
                state[h] = st
                return st

            def emit_z_chunk(c):
                cs = slice(c * CH, (c + 1) * CH)
                ps = pmm.tile([96, CH], F32, tag="mm", name="ps_z")
                for ki in range(KI):
                    nc.tensor.matmul(ps, zbs[:, ki, :], xTs[:, ki, cs],
                                     start=(ki == 0), stop=(ki == KI - 1))
                for pi in range(3):
                    nc.vector.tensor_copy(
                        out=zt[32 * pi:32 * pi + R, cs],
                        in_=ps[32 * pi:32 * pi + R, :])

            def emit_proj_chain(h, p, c):
                """one projection chain: dest[:, chunk] for proj p, head h"""
                st = state[h]
                cs = slice(c * CH, (c + 1) * CH)
                dest = {"q": st["q"], "k": st["k"], "v": st["vT"]}[p]
                ps = pmm.tile([P, CH], F32, tag="mm", name="ps_p")
                ws = st["ws"][p]
                for ki in range(KI):
                    nc.tensor.matmul(ps, ws[:, ki, :], xTs[:, ki, cs],
                                     start=(ki == 0), stop=False)
                pi = "qkv".index(p)
                nc.tensor.matmul(ps, abts[32 * pi:32 * pi + R1, h * P:(h + 1) * P],
                                 zt[32 * pi:32 * pi + R1, cs],
                                 start=False, stop=True)
                nc.vector.tensor_copy(out=dest[:, cs], in_=ps)

            def emit_v_transpose(h, c):
                """transpose the 4 key tiles of chunk c into natural layout"""
                st = state[h]
                for kt in range(4 * c, 4 * c + 4):
                    ps_t = psc.tile([P, P], F32, tag="s", name="ps_t")
                    nc.tensor.matmul(ps_t, st["vT"][:, kt * P:(kt + 1) * P],
                                     ident, start=True, stop=True)
                    # Act engine drains the psum slot fast so the next scores
                    # group is not paced by the DVE backlog
                    nc.scalar.copy(st["v"][:, kt, :], ps_t)

            def emit_group_adds(ex, exs8, exs2, g, pairs_only=False):
                """DVE reduction-tree pieces completed by group g's exps
                (pairs -> quad -> oct). pairs_only trims DVE load for the
                last head, whose DVE is the lagging engine."""
                for j in (4 * g, 4 * g + 2):
                    nc.vector.tensor_add(exs8[:, j // 2, :],
                                         ex[:, j, :], ex[:, j + 1, :])
                if pairs_only:
                    return
                nc.vector.tensor_add(exs8[:, 8 + g, :],
                                     exs8[:, 2 * g, :], exs8[:, 2 * g + 1, :])
                if g in (1, 3):
                    j = g // 2
                    nc.vector.tensor_add(exs2[:, j, :],
                                         exs8[:, 8 + 2 * j, :],
                                         exs8[:, 9 + 2 * j, :])

            def emit_scores_group(h, c, ex, g):
                st = state[h]
                cs = slice(c * CH, (c + 1) * CH)
                # two scores matmuls share a 2-bank psum tile so one exp
                # instruction covers 1024 elements (half the Act overhead)
                for kt0 in (4 * g, 4 * g + 2):
                    ps_s = psc.tile([P, 2, CH], F32, tag="s", name="ps_s")
                    for j in range(2):
                        kt = kt0 + j
                        nc.tensor.matmul(ps_s[:, j, :],
                                         st["k"][:, kt * P:(kt + 1) * P],
                                         st["q"][:, cs], start=True, stop=True)
                    nc.scalar.activation(ex[:, kt0:kt0 + 2, :], ps_s,
                                         AF.Exp, scale=SCALE)

            def emit_attn_rest(h, c, ex, exs8, exs2, filler=None,
                               free_psum_early=False):
                st = state[h]
                cs = slice(c * CH, (c + 1) * CH)
                # attn@v first: it only needs the exp outputs, and its 3.4us
                # give the DVE reduction tree time to finish before the
                # denominator chain consumes it. For the last head (no proj
                # filler) alternate between the idle mm slots and the o slot
                # so each chunk waits on the mul from two chunks ago.
                if h + 1 == HC and c % 2 == 0:
                    ps_o = pmm.tile([P, CH], F32, tag="mm", name="ps_o")
                else:
                    ps_o = pso.tile([P, CH], F32, tag="o", name="ps_o")
                for kt in range(TT):
                    nc.tensor.matmul(ps_o, st["v"][:, kt, :], ex[:, kt, :],
                                     start=(kt == 0), stop=(kt == TT - 1))
                o_src = ps_o
                if free_psum_early:
                    # without proj filler the DVE lags the Act-paced adds,
                    # so the next chunk's attn@v would stall on the mul that
                    # frees this bank; drain it via the Act engine instead
                    o_cp = smpool.tile([P, CH], F32, tag="ocp", name="o_cp",
                                       bufs=1)
                    nc.scalar.copy(o_cp, ps_o)
                    o_src = o_cp
                # group 3's tree adds only now, AFTER the attn@v chain: its
                # order-conservative DVE wait must not include them
                last = h + 1 == HC
                emit_group_adds(ex, exs8, exs2, 3, pairs_only=last)
                # denominators, broadcast across all partitions in one chain;
                # the last head sums its 8 pair tiles on the (idle) PE
                ps_r = psr.tile([P, CH], F32, tag="r", name="ps_r")
                nsum = 8 if last else 2
                dsrc = exs8 if last else exs2
                for j in range(nsum):
                    nc.tensor.matmul(ps_r, ones, dsrc[:, j, :],
                                     start=(j == 0), stop=(j == nsum - 1))
                if filler is not None:
                    filler()
                rb = smpool.tile([P, CH], F32, tag="rb", name="rb", bufs=1)
                nc.vector.reciprocal(out=rb, in_=ps_r)
                nc.vector.tensor_mul(ao[h][:, cs], o_src, rb)

            # ---- prologue: xT chunk 0 rides the Act HWDGE queue in
            # parallel with consts/weights on the sync queue (safe: no
            # matmuls are running yet, so no SBUF contention) ----
            for ki in range(KI):
                dma(xTs[:, ki, 0:CH], xT[ki * P:(ki + 1) * P, 0:CH])
            emit_proj_head_start(0)
            abts = consts.tile([96, DS], BF16, tag="abts")
            dma(abts, abt[:, :])
            for c in range(1, NCH):
                cs = slice(c * CH, (c + 1) * CH)
                for ki in range(KI):
                    dma(xTs[:, ki, cs], xT[ki * P:(ki + 1) * P, cs])
            aoas = consts.tile([R, D], BF16, tag="aoas")
            dma(aoas, aoa[:, :])
            bzs = consts.tile([P, KO, R], BF16, tag="bzs")
            dma(bzs, bz[:, :, :])

            nc.vector.memset(zt[:, :], 1.0)
            for c in range(NCH):
                emit_z_chunk(c)
                for p in "qkv":
                    emit_proj_chain(0, p, c)
                emit_v_transpose(0, c)

            def emit_zo_chunk(c):
                cs = slice(c * CH, (c + 1) * CH)
                ps = pmm.tile([R, CH], F32, tag="mm", name="ps_zo")
                for ki in range(KO):
                    nc.tensor.matmul(ps, bzs[:, ki, :], ao[ki][:, cs],
                                     start=(ki == 0), stop=(ki == KO - 1))
                nc.vector.tensor_copy(out=zos[:, cs], in_=ps)

            # ---- pipelined head loop: attn(h) interleaved with proj(h+1);
            # the last head interleaves the z_o chains instead ----
            for h in range(HC):
                if h + 1 < HC:
                    emit_proj_head_start(h + 1)
                for c in range(NCH):
                    ex = expool.tile([P, TT, CH], BF16, tag="ex", name="ex")
                    exs8 = expool.tile([P, 12, CH], BF16, tag="exs8", name="exs8")
                    exs2 = expool.tile([P, 2, CH], BF16, tag="exs2", name="exs2")
                    lastp = h + 1 == HC
                    emit_scores_group(h, c, ex, 0)
                    emit_group_adds(ex, exs8, exs2, 0, pairs_only=lastp)
                    if h + 1 < HC:
                        emit_proj_chain(h + 1, "q", c)
                    emit_scores_group(h, c, ex, 1)
                    emit_group_adds(ex, exs8, exs2, 1, pairs_only=lastp)
                    if h + 1 < HC:
                        emit_proj_chain(h + 1, "k", c)
                    emit_scores_group(h, c, ex, 2)
                    emit_group_adds(ex, exs8, exs2, 2, pairs_only=lastp)
                    if h + 1 < HC:
                        emit_proj_chain(h + 1, "v", c)
                    emit_scores_group(h, c, ex, 3)
                    filler = None
                    if h + 1 == HC and c > 0:
                        filler = (lambda cc: lambda: emit_zo_chunk(cc))(c - 1)
                    emit_attn_rest(h, c, ex, exs8, exs2, filler)
                    if h + 1 < HC:
                        emit_v_transpose(h + 1, c)
                del state[h]
            emit_zo_chunk(NCH - 1)

            # ---- partial O projection ----
            for do in range(NT):
                wos = wost.tile([P, KO, P], BF16, tag="wo", name="wos")
                dma(wos, wo[do, :, :, :])
                for c in range(NCH):
                    cs = slice(c * CH, (c + 1) * CH)
                    pool = psc if c % 2 == 0 else pso
                    ps = pool.tile([P, CH], F32, tag="s" if c % 2 == 0 else "o",
                                   name="ps_oo")
                    for ki in range(KO):
                        nc.tensor.matmul(ps, wos[:, ki, :], ao[ki][:, cs],
                                         start=(ki == 0), stop=False)
                    nc.tensor.matmul(ps, aoas[:, do * P:(do + 1) * P], zos[:, cs],
                                     start=False, stop=True)
                    o_sb = smpool.tile([P, CH], BF16, tag="osb", name="o_sb", bufs=2)
                    nc.scalar.copy(o_sb, ps)
                    dma(yt[do * P:(do + 1) * P, cs], o_sb)

    nc.compile()
    return nc


def kernel(**inputs):
    inp = {k: np.asarray(v, dtype=np.float32) for k, v in inputs.items()}
    x = inp["x"]

    if "nc" not in _cache:
        _cache["nc"] = _build()
    nc = _cache["nc"]

    def bf(a):
        return np.ascontiguousarray(a).astype(NPBF)

    # host-side packing (shared across the two cores of a batch differs
    # only via the head-half slice)
    halves = []
    for hh in range(2):
        sl = slice(hh * DS, (hh + 1) * DS)
        m = {}
        abt = np.zeros((96, DS), dtype=np.float32)
        for pi, p in enumerate("qkv"):
            W = inp[f"W{p}"]
            # staged [do, p, ki, f] from W^T[:, slice]
            wts = np.ascontiguousarray(W[sl, :].T)           # [D, DS]
            m[f"w{p}"] = bf(wts.reshape(KI, P, HC, P).transpose(2, 1, 0, 3))
            abt[32 * pi:32 * pi + R] = inp[f"A{p}"].T[:, sl]
            abt[32 * pi + R] = inp[f"b{p}"][sl]
        m["abt"] = bf(abt)
        wto = np.ascontiguousarray(inp["Wo"].T)[sl, :]       # [DS, D]
        m["wo"] = bf(wto.reshape(KO, P, NT, P).transpose(2, 1, 0, 3))
        zbc = np.zeros((D, 96), dtype=np.float32)
        for pi, p in enumerate("qkv"):
            zbc[:, 32 * pi:32 * pi + R] = SCALING * inp[f"B{p}"].T
        m["zb"] = bf(zbc.reshape(KI, P, 96).transpose(1, 0, 2))
        bzc = (SCALING * inp["Bo"].T)[sl, :]                 # [DS, R]
        m["bz"] = bf(bzc.reshape(KO, P, R).transpose(1, 0, 2))
        m["aoa"] = bf(inp["Ao"].T)                           # [R, D]
        halves.append(m)

    in_maps = []
    for c in range(NCORES):
        b, hh = c // 2, c % 2
        m = dict(halves[hh])
        m["xT"] = bf(x[b].T)
        in_maps.append(m)

    trace = bool(int(os.environ.get("KERNEL_TRACE", "0")))
    res = run_bass_kernel_spmd(nc, in_maps, list(range(NCORES)), trace=trace)
    _cache["last_exec_time_ns"] = res.exec_time_ns
    _cache["last_result"] = res

    y = np.empty((B, L, D), dtype=np.float32)
    for b in range(B):
        yt0 = res.results[2 * b]["yt"].astype(np.float32)
        yt1 = res.results[2 * b + 1]["yt"].astype(np.float32)
        y[b] = (yt0 + yt1).T + inp["bo"][None, :]
    return y


# revision 78
# speedup vs baseline: 1.0045x; 1.0045x over previous
"""Multi-head self-attention with LoRA on 8 Trainium2 NeuronCores.

Sharding: core c -> (batch b = c//2, head-half = c%2): each core computes
8 of the 16 heads for one batch (tensor parallel over heads), then a
partial O-projection over its 1024 input dims. Host sums the two partial
O outputs per batch and adds the O bias (gather-reduce unshard).

Per core (all SBUF-resident, bf16 matmul inputs, fp32 PSUM accumulate):
  - xT [din, tok] arrives pre-transposed from host (bf16)
  - z = (SCALING*B) @ xT for q/k/v in one combined 48-wide pass
  - per head: qT/kT/vT projections (LoRA + bias folded into the psum
    accumulation chain via an extra rank-17 matmul), v re-transposed to
    natural layout with an identity-moving matmul
  - attention: scores -> exp (Act engine, scale folded) -> ones-matmul
    denominators -> attn@v -> reciprocal-normalize
  - software pipelining: head h's attention is interleaved with head
    h+1's projections so PE never waits on the Act engine's exp
  - partial O-projection [2048, tok] over this core's 1024 dims
Host: input packing/transposes/casts and output pair-sum + bias.
"""

import os
import numpy as np
import ml_dtypes

import concourse.bacc as bacc
import concourse.mybir as mybir
import concourse.tile as tile
from concourse.bass_utils import run_bass_kernel_spmd

F32 = mybir.dt.float32
F32R = mybir.dt.float32r
BF16 = mybir.dt.bfloat16
AF = mybir.ActivationFunctionType
NPBF = ml_dtypes.bfloat16

B, L, D = 4, 2048, 2048
H, HD, R = 16, 128, 16
HC = H // 2            # 8 heads per core
DS = HC * HD           # 1024-dim q/k/v output slice per core
SCALING = 0.5          # lora alpha / rank (folded into B on host)
SCALE = HD ** -0.5     # attention score scale (folded into exp)
P = 128
KI = D // P            # 16 contraction tiles (full model dim)
KO = DS // P           # 8 contraction tiles (this core's O input slice)
NT = D // P            # 16 output tiles for O projection
TT = L // P            # 16 key tiles
CH = 512               # moving-dim chunk
NCH = L // CH          # 4 chunks of query tokens
R1 = R + 1             # lora rank + bias row
NCORES = 8

_cache = {}


def _build():
    nc = bacc.Bacc()

    xT = nc.dram_tensor("xT", [D, L], BF16, kind="ExternalInput")
    # staged weight layouts (packed on host): [do, p, ki, f]
    wq = nc.dram_tensor("wq", [HC, P, KI, P], BF16, kind="ExternalInput")
    wk = nc.dram_tensor("wk", [HC, P, KI, P], BF16, kind="ExternalInput")
    wv = nc.dram_tensor("wv", [HC, P, KI, P], BF16, kind="ExternalInput")
    wo = nc.dram_tensor("wo", [NT, P, KO, P], BF16, kind="ExternalInput")
    # combined scaled B^T for q/k/v, padded to 32-wide blocks: [p, ki, 96]
    zb = nc.dram_tensor("zb", [P, KI, 96], BF16, kind="ExternalInput")
    # scaled B_o^T slice: [p, ki(8), 16]
    bz = nc.dram_tensor("bz", [P, KO, R], BF16, kind="ExternalInput")
    # A^T slices with bias rows, at 32-aligned base partitions per proj:
    # rows 32p..32p+15 = A_p^T slice, row 32p+16 = bias_p slice
    abt = nc.dram_tensor("abt", [96, DS], BF16, kind="ExternalInput")
    aoa = nc.dram_tensor("aoa", [R, D], BF16, kind="ExternalInput")
    yt = nc.dram_tensor("yt", [D, L], BF16, kind="ExternalOutput")

    ident_d = nc.inline_tensor(np.eye(P, dtype=NPBF), name="ident_d")
    ones_d = nc.inline_tensor(np.ones((P, P), dtype=NPBF), name="ones_d")

    def dma(out, in_):
        nc.sync.dma_start(out=out, in_=in_)

    def dma_act(out, in_):
        # second hardware DMA queue (Activation engine HWDGE)
        nc.scalar.dma_start(out=out, in_=in_)

    def fr(ap):
        return ap.bitcast(F32R)

    with tile.TileContext(nc) as tc:
        with (
            tc.tile_pool(name="consts", bufs=1) as consts,
            tc.tile_pool(name="data", bufs=1) as data,
            tc.tile_pool(name="hq", bufs=2) as hqpool,
            tc.tile_pool(name="hk", bufs=2) as hkpool,
            tc.tile_pool(name="hv", bufs=2) as hvpool,
            tc.tile_pool(name="wst", bufs=3) as wst,
            tc.tile_pool(name="wost", bufs=3) as wost,
            tc.tile_pool(name="ex", bufs=1) as expool,
            tc.tile_pool(name="sm", bufs=2) as smpool,
            tc.tile_pool(name="pmm", bufs=2, space="PSUM") as pmm,
            tc.tile_pool(name="psc", bufs=2, space="PSUM") as psc,
            tc.tile_pool(name="pso", bufs=1, space="PSUM") as pso,
            tc.tile_pool(name="psr", bufs=1, space="PSUM") as psr,
        ):
            # ---- constants (only what the z pass needs before xT) ----
            ident = consts.tile([P, P], BF16, tag="ident")
            dma(ident, ident_d[:, :])
            ones = consts.tile([P, P], BF16, tag="ones")
            dma(ones, ones_d[:, :])
            zbs = consts.tile([P, KI, 96], BF16, tag="zbs")
            dma(zbs, zb[:, :, :])

            # lora moving operands: rows 32p..32p+15 = z_p, row 32p+16 = ones
            # (32-aligned base partitions as the PE and BIR verifier require)
            zt = consts.tile([96, L], BF16, tag="zt")
            zos = consts.tile([R, L], BF16, tag="zos")

            # big resident tensors; xT DMA'd chunk-major so the prologue's
            # z/proj chains can start before the whole 8MB has landed
            xTs = data.tile([P, KI, L], BF16, tag="xTs")
            # per-head ao tiles: exact tile-level deps let the z_o chain's
            # first 7 matmuls start before head 7's normalize finishes
            ao = [data.tile([P, L], BF16, tag=f"ao{h}", name=f"ao{h}")
                  for h in range(HC)]

            # ---- per-head state ----
            state = {}

            def emit_proj_head_start(h):
                st = {}
                st["ws"] = {}
                for p, wsrc in (("q", wq), ("k", wk), ("v", wv)):
                    ws = wst.tile([P, KI, P], BF16, tag="w", name=f"ws_{p}")
                    dma(ws, wsrc[h, :, :, :])
                    st["ws"][p] = ws
                st["q"] = hqpool.tile([P, L], BF16, tag="qh", name="qh")
                st["k"] = hkpool.tile([P, L], BF16, tag="kh", name="kh")
                st["vT"] = hvpool.tile([P, L], BF16, tag="vTh", name="vTh")
                st["v"] = hvpool.tile([P, TT, P], BF16, tag="vh", name="vh")
                state[h] = st
                return st

            def emit_z_chunk(c):
                cs = slice(c * CH, (c + 1) * CH)
                ps = pmm.tile([96, CH], F32, tag="mm", name="ps_z")
                for ki in range(KI):
                    nc.tensor.matmul(ps, zbs[:, ki, :], xTs[:, ki, cs],
                                     start=(ki == 0), stop=(ki == KI - 1))
                for pi in range(3):
                    nc.vector.tensor_copy(
                        out=zt[32 * pi:32 * pi + R, cs],
                        in_=ps[32 * pi:32 * pi + R, :])

            def emit_proj_chain(h, p, c):
                """one projection chain: dest[:, chunk] for proj p, head h"""
                st = state[h]
                cs = slice(c * CH, (c + 1) * CH)
                dest = {"q": st["q"], "k": st["k"], "v": st["vT"]}[p]
                ps = pmm.tile([P, CH], F32, tag="mm", name="ps_p")
                ws = st["ws"][p]
                for ki in range(KI):
                    nc.tensor.matmul(ps, ws[:, ki, :], xTs[:, ki, cs],
                                     start=(ki == 0), stop=False)
                pi = "qkv".index(p)
                nc.tensor.matmul(ps, abts[32 * pi:32 * pi + R1, h * P:(h + 1) * P],
                                 zt[32 * pi:32 * pi + R1, cs],
                                 start=False, stop=True)
                nc.vector.tensor_copy(out=dest[:, cs], in_=ps)

            def emit_v_transpose(h, c):
                """transpose the 4 key tiles of chunk c into natural layout"""
                st = state[h]
                for kt in range(4 * c, 4 * c + 4):
                    ps_t = psc.tile([P, P], F32, tag="s", name="ps_t")
                    nc.tensor.matmul(ps_t, st["vT"][:, kt * P:(kt + 1) * P],
                                     ident, start=True, stop=True)
                    # Act engine drains the psum slot fast so the next scores
                    # group is not paced by the DVE backlog
                    nc.scalar.copy(st["v"][:, kt, :], ps_t)

            def emit_group_adds(ex, exs8, exs2, g, pairs_only=False):
                """DVE reduction-tree pieces completed by group g's exps
                (pairs -> quad -> oct). pairs_only trims DVE load for the
                last head, whose DVE is the lagging engine."""
                for j in (4 * g, 4 * g + 2):
                    nc.vector.tensor_add(exs8[:, j // 2, :],
                                         ex[:, j, :], ex[:, j + 1, :])
                if pairs_only:
                    return
                nc.vector.tensor_add(exs8[:, 8 + g, :],
                                     exs8[:, 2 * g, :], exs8[:, 2 * g + 1, :])
                if g in (1, 3):
                    j = g // 2
                    nc.vector.tensor_add(exs2[:, j, :],
                                         exs8[:, 8 + 2 * j, :],
                                         exs8[:, 9 + 2 * j, :])

            def emit_scores_group(h, c, ex, g):
                st = state[h]
                cs = slice(c * CH, (c + 1) * CH)
                # two scores matmuls share a 2-bank psum tile so one exp
                # instruction covers 1024 elements (half the Act overhead)
                for kt0 in (4 * g, 4 * g + 2):
                    ps_s = psc.tile([P, 2, CH], F32, tag="s", name="ps_s")
                    for j in range(2):
                        kt = kt0 + j
                        nc.tensor.matmul(ps_s[:, j, :],
                                         st["k"][:, kt * P:(kt + 1) * P],
                                         st["q"][:, cs], start=True, stop=True)
                    nc.scalar.activation(ex[:, kt0:kt0 + 2, :], ps_s,
                                         AF.Exp, scale=SCALE)

            def emit_attn_rest(h, c, ex, exs8, exs2, filler=None,
                               free_psum_early=False):
                st = state[h]
                cs = slice(c * CH, (c + 1) * CH)
                # attn@v first: it only needs the exp outputs, and its 3.4us
                # give the DVE reduction tree time to finish before the
                # denominator chain consumes it. For the last head (no proj
                # filler) alternate between the idle mm slots and the o slot
                # so each chunk waits on the mul from two chunks ago.
                if h + 1 == HC and c % 2 == 0:
                    ps_o = pmm.tile([P, CH], F32, tag="mm", name="ps_o")
                else:
                    ps_o = pso.tile([P, CH], F32, tag="o", name="ps_o")
                for kt in range(TT):
                    nc.tensor.matmul(ps_o, st["v"][:, kt, :], ex[:, kt, :],
                                     start=(kt == 0), stop=(kt == TT - 1))
                o_src = ps_o
                if free_psum_early:
                    # without proj filler the DVE lags the Act-paced adds,
                    # so the next chunk's attn@v would stall on the mul that
                    # frees this bank; drain it via the Act engine instead
                    o_cp = smpool.tile([P, CH], F32, tag="ocp", name="o_cp",
                                       bufs=1)
                    nc.scalar.copy(o_cp, ps_o)
                    o_src = o_cp
                # group 3's tree adds only now, AFTER the attn@v chain: its
                # order-conservative DVE wait must not include them
                emit_group_adds(ex, exs8, exs2, 3)
                # denominators, broadcast across all partitions in one chain
                ps_r = psr.tile([P, CH], F32, tag="r", name="ps_r")
                for j in range(2):
                    nc.tensor.matmul(ps_r, ones, exs2[:, j, :],
                                     start=(j == 0), stop=(j == 1))
                if filler is not None:
                    filler()
                rb = smpool.tile([P, CH], F32, tag="rb", name="rb", bufs=1)
                nc.vector.reciprocal(out=rb, in_=ps_r)
                nc.vector.tensor_mul(ao[h][:, cs], o_src, rb)

            # ---- prologue: xT chunk 0 rides the Act HWDGE queue in
            # parallel with consts/weights on the sync queue (safe: no
            # matmuls are running yet, so no SBUF contention) ----
            for ki in range(KI):
                dma_act(xTs[:, ki, 0:CH], xT[ki * P:(ki + 1) * P, 0:CH])
            emit_proj_head_start(0)
            abts = consts.tile([96, DS], BF16, tag="abts")
            dma(abts, abt[:, :])
            for c in range(1, NCH):
                cs = slice(c * CH, (c + 1) * CH)
                for ki in range(KI):
                    dma(xTs[:, ki, cs], xT[ki * P:(ki + 1) * P, cs])
            aoas = consts.tile([R, D], BF16, tag="aoas")
            dma(aoas, aoa[:, :])
            bzs = consts.tile([P, KO, R], BF16, tag="bzs")
            dma(bzs, bz[:, :, :])

            nc.vector.memset(zt[:, :], 1.0)
            for c in range(NCH):
                emit_z_chunk(c)
                for p in "qkv":
                    emit_proj_chain(0, p, c)
                emit_v_transpose(0, c)

            def emit_zo_chunk(c):
                cs = slice(c * CH, (c + 1) * CH)
                ps = pmm.tile([R, CH], F32, tag="mm", name="ps_zo")
                for ki in range(KO):
                    nc.tensor.matmul(ps, bzs[:, ki, :], ao[ki][:, cs],
                                     start=(ki == 0), stop=(ki == KO - 1))
                nc.vector.tensor_copy(out=zos[:, cs], in_=ps)

            wos0 = [None]

            def emit_o_fill(c):
                """PE filler for the last head: O-projection do-tile 0 for
                chunk c-2 (its z_o chunk is ready by then)"""
                if c < 2:
                    return
                cp = c - 2
                cs = slice(cp * CH, (cp + 1) * CH)
                ps = pmm.tile([P, CH], F32, tag="mm", name="ps_of")
                for ki in range(KO):
                    nc.tensor.matmul(ps, wos0[0][:, ki, :], ao[ki][:, cs],
                                     start=(ki == 0), stop=False)
                nc.tensor.matmul(ps, aoas[:, 0:P], zos[:, cs],
                                 start=False, stop=True)
                o_sb = smpool.tile([P, CH], BF16, tag="osb", name="o_sb",
                                   bufs=2)
                nc.scalar.copy(o_sb, ps)
                dma(yt[0:P, cs], o_sb)

            # ---- pipelined head loop: attn(h) interleaved with proj(h+1);
            # the last head interleaves the z_o chains instead ----
            for h in range(HC):
                if h + 1 < HC:
                    emit_proj_head_start(h + 1)
                else:
                    wos0[0] = wost.tile([P, KO, P], BF16, tag="wo",
                                        name="wos0")
                    dma(wos0[0], wo[0, :, :, :])
                for c in range(NCH):
                    ex = expool.tile([P, TT, CH], BF16, tag="ex", name="ex")
                    exs8 = expool.tile([P, 12, CH], BF16, tag="exs8", name="exs8")
                    exs2 = expool.tile([P, 2, CH], BF16, tag="exs2", name="exs2")
                    emit_scores_group(h, c, ex, 0)
                    emit_group_adds(ex, exs8, exs2, 0)
                    if h + 1 < HC:
                        emit_proj_chain(h + 1, "q", c)
                    else:
                        emit_o_fill(c)
                    emit_scores_group(h, c, ex, 1)
                    emit_group_adds(ex, exs8, exs2, 1)
                    if h + 1 < HC:
                        emit_proj_chain(h + 1, "k", c)
                    emit_scores_group(h, c, ex, 2)
                    emit_group_adds(ex, exs8, exs2, 2)
                    if h + 1 < HC:
                        emit_proj_chain(h + 1, "v", c)
                    emit_scores_group(h, c, ex, 3)
                    filler = None
                    if h + 1 == HC and c > 0:
                        filler = (lambda cc: lambda: emit_zo_chunk(cc))(c - 1)
                    emit_attn_rest(h, c, ex, exs8, exs2, filler)
                    if h + 1 < HC:
                        emit_v_transpose(h + 1, c)
                del state[h]
            emit_zo_chunk(NCH - 1)

            # ---- partial O projection ----
            for do in range(NT):
                wos = wost.tile([P, KO, P], BF16, tag="wo", name="wos")
                dma(wos, wo[do, :, :, :])
                for c in range(NCH):
                    if do == 0 and c < 2:
                        continue  # emitted as head-7 o-fill
                    cs = slice(c * CH, (c + 1) * CH)
                    pool = psc if c % 2 == 0 else pso
                    ps = pool.tile([P, CH], F32, tag="s" if c % 2 == 0 else "o",
                                   name="ps_oo")
                    for ki in range(KO):
                        nc.tensor.matmul(ps, wos[:, ki, :], ao[ki][:, cs],
                                         start=(ki == 0), stop=False)
                    nc.tensor.matmul(ps, aoas[:, do * P:(do + 1) * P], zos[:, cs],
                                     start=False, stop=True)
                    o_sb = smpool.tile([P, CH], BF16, tag="osb", name="o_sb", bufs=2)
                    nc.scalar.copy(o_sb, ps)
                    dma(yt[do * P:(do + 1) * P, cs], o_sb)

    nc.compile()
    return nc


def kernel(**inputs):
    inp = {k: np.asarray(v, dtype=np.float32) for k, v in inputs.items()}
    x = inp["x"]

    if "nc" not in _cache:
        _cache["nc"] = _build()
    nc = _cache["nc"]

    def bf(a):
        return np.ascontiguousarray(a).astype(NPBF)

    # host-side packing (shared across the two cores of a batch differs
    # only via the head-half slice)
    halves = []
    for hh in range(2):
        sl = slice(hh * DS, (hh + 1) * DS)
        m = {}
        abt = np.zeros((96, DS), dtype=np.float32)
        for pi, p in enumerate("qkv"):
            W = inp[f"W{p}"]
            # staged [do, p, ki, f] from W^T[:, slice]
            wts = np.ascontiguousarray(W[sl, :].T)           # [D, DS]
            m[f"w{p}"] = bf(wts.reshape(KI, P, HC, P).transpose(2, 1, 0, 3))
            abt[32 * pi:32 * pi + R] = inp[f"A{p}"].T[:, sl]
            abt[32 * pi + R] = inp[f"b{p}"][sl]
        m["abt"] = bf(abt)
        wto = np.ascontiguousarray(inp["Wo"].T)[sl, :]       # [DS, D]
        m["wo"] = bf(wto.reshape(KO, P, NT, P).transpose(2, 1, 0, 3))
        zbc = np.zeros((D, 96), dtype=np.float32)
        for pi, p in enumerate("qkv"):
            zbc[:, 32 * pi:32 * pi + R] = SCALING * inp[f"B{p}"].T
        m["zb"] = bf(zbc.reshape(KI, P, 96).transpose(1, 0, 2))
        bzc = (SCALING * inp["Bo"].T)[sl, :]                 # [DS, R]
        m["bz"] = bf(bzc.reshape(KO, P, R).transpose(1, 0, 2))
        m["aoa"] = bf(inp["Ao"].T)                           # [R, D]
        halves.append(m)

    in_maps = []
    for c in range(NCORES):
        b, hh = c // 2, c % 2
        m = dict(halves[hh])
        m["xT"] = bf(x[b].T)
        in_maps.append(m)

    trace = bool(int(os.environ.get("KERNEL_TRACE", "0")))
    res = run_bass_kernel_spmd(nc, in_maps, list(range(NCORES)), trace=trace)
    _cache["last_exec_time_ns"] = res.exec_time_ns
    _cache["last_result"] = res

    y = np.empty((B, L, D), dtype=np.float32)
    for b in range(B):
        yt0 = res.results[2 * b]["yt"].astype(np.float32)
        yt1 = res.results[2 * b + 1]["yt"].astype(np.float32)
        y[b] = (yt0 + yt1).T + inp["bo"][None, :]
    return y
